# revision 26
# baseline (speedup 1.0000x reference)
"""MoE adapter (top-1 of 4 experts, dense all-expert reference) on 8 TRN2 NeuronCores.

Strategy
--------
Data-parallel over the 32768 tokens (4096 per core); expert weights replicated.

The reference computes every expert's bottleneck MLP (D=768 -> H=192 -> D=768)
on all tokens and combines with the one-hot top-1 dispatch mask.  Since
4 experts x H=192 = 768, the four expert MLPs stack into two dense 768x768
matmuls:

    h_all = gelu(x @ W1_stacked + b1_stacked)       # [T, 768]
    mh    = h_all * expand(one_hot)                  # zero non-selected blocks
    y     = mh @ W2_stacked + one_hot @ b2           # [T, 768]
    out   = y + x

Key device-side choices (all tuned against perfetto traces):
  * one fp16 feature-major stream of 16*x feeds router, mm1 precision
    (via an fp8 shadow), and the skip connection; output ships feature-major
    (host untransposes), so no token-major stream exists at all.
  * mm2 keeps W2 stationary (resident in SBUF -> every LDWEIGHTS pulled
    ahead by the PE reorder window), mh moves, y^T comes out feature-major.
  * the one-hot mask is applied to mm1's PSUM *before* the gelu (DVE
    tensor_tensor on the fast path; a direct fp16*fp16->fp8 mask measured
    1.6us vs 0.66us this way), and the gelu writes fp8 mh directly.
    Masked units then hold gelu(b1) constants; their (routing-dependent)
    contribution through W2 is cancelled exactly by a host-side adjustment
    of b2 (see _prep_inputs).
  * every matmul is a full (128,128)-tile op - the router output is
    zero-padded to M=128, the one-hot psm expansion is K=128 against
    zero-padded one-hot tiles, and b2 is injected as a K=128 matmul
    against the already-expanded mask (b2big[p,:] = b2adj[p%4,:]/32) -
    so the PE never switches tiling modes.
  * the skip add rides the DMA engines: a gpsimd SWDGE SBUF->SBUF
    transfer accumulates x^T into the drained y^T tile (CCE add),
    freeing the DVE for the mask work.

Scaling: x ships as 16*x (argmax is scale invariant given 16*router_bias),
W1/W2/b2 ship x16 (fp8 subnormal safety), so mm1 psum = 256*(x@w1)
(descaled inside the gelu), mm2 psum = 16*y, out = 16*(y+x) fp16, host /16.
"""

import math

import numpy as np
import ml_dtypes

import concourse.bass as bass
import concourse.mybir as mybir
import concourse.tile as tile
from concourse import bacc
from concourse.bass_utils import run_bass_kernel_spmd

F16 = np.float16
F32 = np.float32

B, S, D = 16, 2048, 768
H, E = 192, 4
N_CORES = 8
TOK_TOTAL = B * S                 # 32768
TOK = TOK_TOTAL // N_CORES        # 4096 tokens per core
TILE = 512                        # tokens per pipeline tile
N_TILES = TOK // TILE             # 8
KC = D // 128                     # 6 contraction chunks

_NC_CACHE = None


def _build_bass():
    dt = mybir.dt
    nc = bacc.Bacc("TRN2", target_bir_lowering=False)

    xht = nc.dram_tensor("xht", [N_TILES, 128, KC * TILE], dt.float16, kind="ExternalInput")
    xh8d = nc.dram_tensor("xh8d", [N_TILES, 128, KC * TILE], dt.float8e4, kind="ExternalInput")
    w1s = nc.dram_tensor("w1s", [128, KC * D], dt.float8e4, kind="ExternalInput")
    w2s = nc.dram_tensor("w2s", [128, KC * D], dt.float8e4, kind="ExternalInput")
    rwp = nc.dram_tensor("rwp", [128, KC * 128], dt.float16, kind="ExternalInput")
    eep = nc.dram_tensor("eep", [128, 128], dt.float16, kind="ExternalInput")
    b2p = nc.dram_tensor("b2p", [128, D], dt.float16, kind="ExternalInput")
    b1r = nc.dram_tensor("b1r", [128, KC], dt.float32, kind="ExternalInput")
    rbt = nc.dram_tensor("rbt", [32, E], dt.float32, kind="ExternalInput")
    out = nc.dram_tensor("out", [N_TILES, 128, KC * TILE], dt.float16, kind="ExternalOutput")

    # feature-major x views, pre-tiled partition-contiguous
    xht_r = xht.rearrange("n p (c t) -> n p c t", c=KC)
    xh8_r = xh8d.rearrange("n p (c t) -> n p c t", c=KC)
    out_r = out.rearrange("n p (c t) -> n p c t", c=KC)

    add = mybir.AluOpType.add
    mult = mybir.AluOpType.mult
    amax = mybir.AluOpType.max
    iseq = mybir.AluOpType.is_equal

    with tile.TileContext(nc) as tc:
        with (
            tc.tile_pool(name="const", bufs=1) as const,
            tc.tile_pool(name="xin", bufs=3) as xin,
            tc.tile_pool(name="hbuf", bufs=3) as hbuf,
            tc.tile_pool(name="obuf", bufs=2) as obuf,
            tc.tile_pool(name="small", bufs=3) as small,
            tc.tile_pool(name="ps_rm", bufs=1, space="PSUM") as ps_rm,
            tc.tile_pool(name="ps_h", bufs=2, space="PSUM") as ps_h,
            tc.tile_pool(name="ps_y", bufs=5, space="PSUM") as ps_y,
        ):
            # ---- constants.  The router weight rides the sync HWDGE FIFO
            # FIRST (it gates tile-0's router matmuls); the other small
            # constants go on the scalar (ACT) HWDGE ring which is otherwise
            # idle during the DMA head.
            rwsb = const.tile([128, KC, 128], dt.float16)
            nc.sync.dma_start(rwsb, rwp.rearrange("p (c m) -> p c m", c=KC))
            eesb = const.tile([128, 128], dt.float16)
            nc.scalar.dma_start(eesb, eep[:])
            rbsb = const.tile([32, E], dt.float32)
            nc.scalar.dma_start(rbsb, rbt[:])
            b1sb = const.tile([128, KC], dt.float32)
            nc.scalar.dma_start(b1sb, b1r[:])
            b2sb = const.tile([128, D], dt.float16)
            nc.scalar.dma_start(b2sb, b2p[:])
            w1sb = const.tile([128, KC, D], dt.float8e4)
            w2sb = const.tile([128, KC, D], dt.float8e4)

            def load_tiles(it):
                xh = xin.tile([128, KC, TILE], dt.float16, tag="xh")
                nc.sync.dma_start(xh, xht_r[it])
                if it == 0:
                    nc.sync.dma_start(w1sb, w1s.rearrange("p (c h) -> p c h", c=KC))
                xh8 = xin.tile([128, KC, TILE], dt.float8e4, tag="xh8")
                nc.sync.dma_start(xh8, xh8_r[it])
                if it == 1:
                    nc.sync.dma_start(w2sb, w2s.rearrange("p (c h) -> p c h", c=KC))
                return xh, xh8

            def router_logits(xh):
                """router matmuls only (PE side), zero-padded to M=128."""
                psrm = ps_rm.tile([128, TILE], dt.float32, tag="psrm")
                for kc in range(KC):
                    nc.tensor.matmul(
                        psrm, rwsb[:, kc, :], xh[:, kc, :],
                        start=(kc == 0), stop=(kc == KC - 1),
                    )
                return psrm

            def router_onehot(psrm):
                """logits^T in psum -> one-hot mask mt32[0:4] [4, TILE] fp16.

                mt32 is a [128, TILE] tile whose rows 4:128 are kept zero
                (prologue memset) so the K=128 psm expansion never sees
                garbage.  Emitted *after* the owning tile's skip adds so the
                DVE drains mm2's psum banks before starting this chain.
                """
                # alignment-safe one-hot argmax via DVE 32x32 stream transpose
                lt32s = small.tile([32, TILE], dt.float32, tag="lt32s")
                nc.scalar.copy(lt32s[0:4], psrm[0:4])
                # token-major blocks: lt32[p, 32g+r] = lt32s[r, 32g+p]
                lt32 = small.tile([32, TILE], dt.float32, tag="lt32")
                nc.vector.transpose(lt32, lt32s)
                v = lt32.rearrange("p (g r) -> p g r", r=32)
                lt_tok = small.tile([32, TILE // 32, E], dt.float32, tag="lt_tok")
                nc.vector.tensor_tensor(
                    lt_tok, v[:, :, 0:E],
                    rbsb[:, None, :].to_broadcast((32, TILE // 32, E)), add,
                )
                mxg = small.tile([32, TILE // 32], dt.float32, tag="mxg")
                nc.vector.tensor_reduce(
                    out=mxg, in_=lt_tok, axis=mybir.AxisListType.X, op=amax
                )
                mtb = small.tile([32, TILE], dt.float16, tag="mtb")
                mview = mtb.rearrange("p (g r) -> p g r", r=32)
                nc.vector.tensor_tensor(
                    mview[:, :, 0:4], lt_tok,
                    mxg[:, :, None].to_broadcast((32, TILE // 32, E)), iseq,
                )
                # back-transpose: mt32[e, t] = one_hot[t, e] for e < 4
                mt32 = small.tile([128, TILE], dt.float16, tag="mt32")
                nc.vector.transpose(mt32[0:32], mtb)
                return mt32

            # ---- prologue: spin the PE so the HAM reaches K=8/8 during the
            # DMA head, then zero the one-hot staging tiles once (their
            # padding rows are never written again).
            dummy = const.tile([128, TILE], dt.float16)
            nc.vector.memset(dummy, 0.0)
            psd = ps_h.tile([128, TILE], dt.float32, tag="psh")
            for _ in range(12):
                nc.tensor.matmul(psd, dummy[:, 0:128], dummy, start=True, stop=True)
            for _ in range(3):
                mtb0 = small.tile([32, TILE], dt.float16, tag="mtb")
                nc.vector.memset(mtb0, 0.0)
                mt0 = small.tile([128, TILE], dt.float16, tag="mt32")
                nc.vector.memset(mt0, 0.0)

            # software pipeline: router matmuls for tile n+1 issue between
            # mm1 and mm2 of tile n (PE), but their DVE argmax chain is
            # emitted after tile n's skip adds; loads run two tiles ahead.
            def psm_expand(psrm, mt32):
                """one-hot -> [128, TILE] mask in SBUF, reusing the router's
                psum bank (its logit rows were already copied to lt32s)."""
                nc.tensor.matmul(psrm, eesb, mt32, start=True, stop=True)
                psm_sb = hbuf.tile([128, TILE], dt.float16, tag="psm_sb")
                nc.scalar.copy(psm_sb, psrm)
                return psm_sb

            tiles = {0: load_tiles(0)}
            psrm0 = router_logits(tiles[0][0])
            psm_sb = psm_expand(psrm0, router_onehot(psrm0))
            tiles[1] = load_tiles(1)

            for it in range(N_TILES):
                xh, xh8 = tiles[it]

                if it + 2 < N_TILES:
                    tiles[it + 2] = load_tiles(it + 2)

                # ---- mm1: pshm = (W1^T x) * mask, mh = gelu(pshm/256 + b1)
                # experts are interleaved along H (unit j of expert e at
                # 4j+e), so the expanded one-hot (psm_sb, produced at the
                # tail of the previous tile) is one [128, TILE] tile for
                # every H-chunk.
                mh = hbuf.tile([128, KC, TILE], dt.float8e4, tag="mh")
                for hc in range(KC):
                    psh = ps_h.tile([128, TILE], dt.float32, tag="psh")
                    for k2 in range(KC // 2):
                        nc.tensor.matmul(
                            psh,
                            w1sb[:, 2 * k2 : 2 * k2 + 2, hc * 128 : (hc + 1) * 128],
                            xh8[:, 2 * k2 : 2 * k2 + 2, :],
                            start=(k2 == 0), stop=(k2 == KC // 2 - 1),
                            perf_mode=mybir.MatmulPerfMode.DoubleRow,
                        )
                    pshm = hbuf.tile([128, TILE], dt.float16, tag="pshm")
                    nc.vector.tensor_tensor(pshm, psh, psm_sb, mult)
                    # x and W1 both shipped x16: descale 1/256 here.  Masked
                    # units emit gelu(b1) constants - cancelled via b2p.
                    nc.scalar.activation(
                        mh[:, hc, :], pshm,
                        mybir.ActivationFunctionType.Gelu,
                        bias=b1sb[:, hc : hc + 1], scale=1.0 / 256.0,
                    )

                # router matmuls for the next tile sit between mm1's and
                # mm2's matmuls in the PE queue; their DVE argmax chain is
                # emitted right behind them so it runs between this tile's
                # pmasks and skip adds - finishing well before the next
                # tile's psm matmul needs mt32 (v4's tail-emitted chain made
                # the psm matmul stall ~2us at every tile boundary).
                if it + 1 < N_TILES:
                    psrm_next = router_logits(tiles[it + 1][0])
                    mt32_next = router_onehot(psrm_next)

                # ---- mm2: y^T = W2s^T @ mh + b2 via the expanded mask,
                # feature-major.  W2 chunks are stationary (resident), mh
                # streams; b2big[p,:] = b2adj[p%4,:]/32 so the K=128 matmul
                # against psm_sb injects exactly b2adj[e(t),:].
                osb = obuf.tile([128, KC, TILE], dt.float16, tag="osb")
                for dc in range(KC):
                    psy = ps_y.tile([128, TILE], dt.float32, tag="psy")
                    nc.tensor.matmul(
                        psy, b2sb[:, dc * 128 : (dc + 1) * 128], psm_sb,
                        start=True, stop=False,
                    )
                    for k2 in range(KC // 2):
                        nc.tensor.matmul(
                            psy,
                            w2sb[:, 2 * k2 : 2 * k2 + 2, dc * 128 : (dc + 1) * 128],
                            mh[:, 2 * k2 : 2 * k2 + 2, :],
                            start=False, stop=(k2 == KC // 2 - 1),
                            perf_mode=mybir.MatmulPerfMode.DoubleRow,
                        )
                    # psum drain on ACT (slack engine), then the skip add as
                    # a cheap all-fp16 DVE op (2x mode; a direct psum-source
                    # add is 661ns vs 327ns this way, and the DVE paces the
                    # tile tail)
                    ycop = hbuf.tile([128, TILE], dt.float16, tag="ycop")
                    nc.scalar.copy(ycop, psy)
                    nc.vector.tensor_tensor(
                        osb[:, dc, :], ycop, xh[:, dc, :], add,
                    )
                    if dc % 2 == 1:
                        nc.sync.dma_start(
                            out_r[it, :, dc - 1 : dc + 1, :],
                            osb[:, dc - 1 : dc + 1, :],
                        )

                # the next tile's mask expansion runs at this tile's tail:
                # its argmax chain is long done, and the ACT copy overlaps
                # the boundary so neither b2 nor the pmasks ever wait on it.
                if it + 1 < N_TILES:
                    psm_sb = psm_expand(psrm_next, mt32_next)
                del tiles[it]

    nc.compile()
    return nc


def _prep_inputs(x, router_w, router_b, w1, b1, w2, b2):
    """Host-side packing: cast/transpose; returns per-core input dicts."""
    xf = np.ascontiguousarray(np.asarray(x, dtype=F32).reshape(TOK_TOTAL, D))

    rw = np.asarray(router_w, dtype=F32)
    w1f = np.asarray(w1, dtype=F32)           # [E, D, H]
    w2f = np.asarray(w2, dtype=F32)           # [E, H, D]
    b1f = np.asarray(b1, dtype=F32)           # [E, H]
    b2f = np.asarray(b2, dtype=F32)           # [E, D]
    rb = np.asarray(router_b, dtype=F32)      # [E]
    F8 = ml_dtypes.float8_e4m3

    # router weight, zero-padded to M=128 outputs, partition-contiguous
    rwp = np.zeros((128, KC, 128), dtype=F16)
    rwp[:, :, 0:E] = rw.astype(F16).reshape(KC, 128, E).transpose(1, 0, 2)
    rwp = np.ascontiguousarray(rwp.reshape(128, KC * 128))

    # experts interleaved along the stacked hidden dim: unit j of expert e
    # lives at index 4j + e  -> the one-hot expansion pattern repeats every
    # 4 partitions, identically for each 128-row chunk.
    # Weights pre-arranged partition-contiguous: [p, c*D+m] = W[(c*128+p), m]
    w1st = w1f.transpose(1, 2, 0).reshape(D, H * E).astype(F16)
    w2st = w2f.transpose(1, 0, 2).reshape(H * E, D).astype(F16)
    w1s = np.ascontiguousarray(
        (w1st.astype(F32) * 16.0)
        .reshape(KC, 128, D).transpose(1, 0, 2).reshape(128, KC * D)).astype(F8)
    w2s = np.ascontiguousarray(
        (w2st.astype(F32) * 16.0)
        .reshape(KC, 128, D).transpose(1, 0, 2).reshape(128, KC * D)).astype(F8)
    b1all = np.ascontiguousarray(b1f.T.reshape(E * H))                    # [768]
    b1r = np.ascontiguousarray(b1all.reshape(KC, 128).T).astype(F32)      # [128, 6]
    rbt = np.ascontiguousarray(np.tile(16.0 * rb.reshape(1, E), (32, 1))).astype(F32)

    # zero-padded one-hot expansion stationary: psm[c,t] = [c%4 == e(t)]
    eep = np.zeros((128, 128), dtype=F16)
    for e in range(E):
        eep[e, e::E] = 1

    # b2 adjusted to cancel the masked units' fp8(gelu(b1)) constants:
    # device adds sum_{u masked} g8[u] w2q[u,:] = C_all - C_e per token.
    u = np.arange(H * E)
    erf1 = np.vectorize(math.erf)
    g8 = (0.5 * b1all * (1.0 + erf1(b1all / np.sqrt(2.0)))).astype(F8).astype(F32)
    w2q = w2s.reshape(128, KC, D).transpose(1, 0, 2).reshape(H * E, D).astype(F32)
    C = np.stack([(g8[u % E == e, None] * w2q[u % E == e]).sum(0) for e in range(E)])
    b2adj = 16.0 * b2f - C.sum(0)[None, :] + C                            # [E, D]
    # replicated across partitions for the K=128 injection via psm_sb
    b2p = np.ascontiguousarray(np.tile(b2adj / 32.0, (128 // E, 1))).astype(F16)

    in_maps = []
    for c in range(N_CORES):
        sl = slice(c * TOK, (c + 1) * TOK)
        xc = xf[sl]
        # feature-major 16*x stream, pre-tiled: [tile, p, (c t)]
        xht_t = np.ascontiguousarray(
            (xc * 16.0).astype(F16)
            .T.reshape(KC, 128, N_TILES, TILE)
            .transpose(2, 1, 0, 3).reshape(N_TILES, 128, KC * TILE))
        xh8_t = np.ascontiguousarray(xht_t.astype(F8))
        in_maps.append(
            {
                "xht": xht_t,
                "xh8d": xh8_t,
                "w1s": w1s,
                "w2s": w2s,
                "rwp": rwp,
                "eep": eep,
                "b2p": b2p,
                "b1r": b1r,
                "rbt": rbt,
            }
        )
    return in_maps


def _get_nc():
    global _NC_CACHE
    if _NC_CACHE is None:
        _NC_CACHE = _build_bass()
    return _NC_CACHE


def kernel(x, router_w, router_b, w1, b1, w2, b2, _trace=False, _trace_kwargs=None):
    in_maps = _prep_inputs(x, router_w, router_b, w1, b1, w2, b2)
    nc = _get_nc()
    res = run_bass_kernel_spmd(
        nc,
        in_maps,
        core_ids=list(range(N_CORES)),
        trace=_trace,
        **(_trace_kwargs or {}),
    )
    outs = []
    for r in res.results:
        o = r["out"].reshape(N_TILES, 128, KC, TILE)     # [it, p, dc, t] fp16
        # y^T feature-major -> token-major [TOK, D], descale by 16
        yt = o.transpose(0, 3, 2, 1).reshape(TOK, D)     # [it*t, dc*128(=d)]
        outs.append(yt.astype(np.float32) * (1.0 / 16.0))
    full = np.concatenate(outs, axis=0).reshape(B, S, D)
    if _trace:
        kernel.last_results = res
    return full


# revision 27
# speedup vs baseline: 1.0869x; 1.0869x over previous
"""MoE adapter (top-1 of 4 experts, dense all-expert reference) on 8 TRN2 NeuronCores.

Strategy
--------
Data-parallel over the 32768 tokens (4096 per core); expert weights replicated.

The reference computes every expert's bottleneck MLP (D=768 -> H=192 -> D=768)
on all tokens and combines with the one-hot top-1 dispatch mask.  Since
4 experts x H=192 = 768, the four expert MLPs stack into two dense 768x768
matmuls:

    h_all = gelu(x @ W1_stacked + b1_stacked)       # [T, 768]
    mh    = h_all * expand(one_hot)                  # zero non-selected blocks
    y     = mh @ W2_stacked + one_hot @ b2           # [T, 768]
    out   = y + x

Key device-side choices (all tuned against perfetto traces):
  * one fp16 feature-major stream of 16*x feeds router, mm1 precision
    (via an fp8 shadow), and the skip connection; output ships feature-major
    (host untransposes), so no token-major stream exists at all.
  * mm2 keeps W2 stationary (resident in SBUF -> every LDWEIGHTS pulled
    ahead by the PE reorder window), mh moves, y^T comes out feature-major.
  * the one-hot mask is applied to mm1's PSUM *before* the gelu (DVE
    tensor_tensor on the fast path; a direct fp16*fp16->fp8 mask measured
    1.6us vs 0.66us this way), and the gelu writes fp8 mh directly.
    Masked units then hold gelu(b1) constants; their (routing-dependent)
    contribution through W2 is cancelled exactly by a host-side adjustment
    of b2 (see _prep_inputs).
  * every matmul is a full (128,128)-tile op - the router output is
    zero-padded to M=128, the one-hot psm expansion is K=128 against
    zero-padded one-hot tiles, and b2 is injected as a K=128 matmul
    against the already-expanded mask (b2big[p,:] = b2adj[p%4,:]/32) -
    so the PE never switches tiling modes.
  * the skip add rides the DMA engines: a gpsimd SWDGE SBUF->SBUF
    transfer accumulates x^T into the drained y^T tile (CCE add),
    freeing the DVE for the mask work.

Scaling: x ships as 16*x (argmax is scale invariant given 16*router_bias),
W1/W2/b2 ship x16 (fp8 subnormal safety), so mm1 psum = 256*(x@w1)
(descaled inside the gelu), mm2 psum = 16*y, out = 16*(y+x) fp16, host /16.
"""

import math

import numpy as np
import ml_dtypes

import concourse.bass as bass
import concourse.mybir as mybir
import concourse.tile as tile
from concourse import bacc
from concourse.bass_utils import run_bass_kernel_spmd

F16 = np.float16
F32 = np.float32

B, S, D = 16, 2048, 768
H, E = 192, 4
N_CORES = 8
TOK_TOTAL = B * S                 # 32768
TOK = TOK_TOTAL // N_CORES        # 4096 tokens per core
TILE = 512                        # tokens per pipeline tile
N_TILES = TOK // TILE             # 8
KC = D // 128                     # 6 contraction chunks

_NC_CACHE = None


def _build_bass():
    dt = mybir.dt
    nc = bacc.Bacc("TRN2", target_bir_lowering=False)

    xht = nc.dram_tensor("xht", [N_TILES, 128, KC * TILE], dt.float16, kind="ExternalInput")
    xh8d = nc.dram_tensor("xh8d", [N_TILES, 128, KC * TILE], dt.float8e4, kind="ExternalInput")
    w1s = nc.dram_tensor("w1s", [128, KC * D], dt.float8e4, kind="ExternalInput")
    w2s = nc.dram_tensor("w2s", [128, KC * D], dt.float8e4, kind="ExternalInput")
    rwp = nc.dram_tensor("rwp", [128, KC * 128], dt.float16, kind="ExternalInput")
    eep = nc.dram_tensor("eep", [128, 128], dt.float16, kind="ExternalInput")
    b2p = nc.dram_tensor("b2p", [128, D], dt.float16, kind="ExternalInput")
    b1r = nc.dram_tensor("b1r", [128, KC], dt.float32, kind="ExternalInput")
    rbt = nc.dram_tensor("rbt", [32, E], dt.float32, kind="ExternalInput")
    out = nc.dram_tensor("out", [N_TILES, 128, KC * TILE], dt.float16, kind="ExternalOutput")

    # feature-major x views, pre-tiled partition-contiguous
    xht_r = xht.rearrange("n p (c t) -> n p c t", c=KC)
    xh8_r = xh8d.rearrange("n p (c t) -> n p c t", c=KC)
    out_r = out.rearrange("n p (c t) -> n p c t", c=KC)

    add = mybir.AluOpType.add
    mult = mybir.AluOpType.mult
    amax = mybir.AluOpType.max
    iseq = mybir.AluOpType.is_equal

    with tile.TileContext(nc) as tc:
        with (
            tc.tile_pool(name="const", bufs=1) as const,
            tc.tile_pool(name="xin", bufs=3) as xin,
            tc.tile_pool(name="hbuf", bufs=3) as hbuf,
            tc.tile_pool(name="obuf", bufs=2) as obuf,
            tc.tile_pool(name="small", bufs=3) as small,
            tc.tile_pool(name="ps_rm", bufs=1, space="PSUM") as ps_rm,
            tc.tile_pool(name="ps_h", bufs=2, space="PSUM") as ps_h,
            tc.tile_pool(name="ps_y", bufs=5, space="PSUM") as ps_y,
        ):
            # ---- constants.  The router weight rides the sync HWDGE FIFO
            # FIRST (it gates tile-0's router matmuls); the other small
            # constants go on the scalar (ACT) HWDGE ring which is otherwise
            # idle during the DMA head.
            rwsb = const.tile([128, KC, 128], dt.float16)
            nc.sync.dma_start(rwsb, rwp.rearrange("p (c m) -> p c m", c=KC))
            eesb = const.tile([128, 128], dt.float16)
            nc.scalar.dma_start(eesb, eep[:])
            rbsb = const.tile([32, E], dt.float32)
            nc.scalar.dma_start(rbsb, rbt[:])
            b1sb = const.tile([128, KC], dt.float32)
            nc.scalar.dma_start(b1sb, b1r[:])
            b2sb = const.tile([128, D], dt.float16)
            nc.scalar.dma_start(b2sb, b2p[:])
            w1sb = const.tile([128, KC, D], dt.float8e4)
            w2sb = const.tile([128, KC, D], dt.float8e4)

            def load_tiles(it):
                xh = xin.tile([128, KC, TILE], dt.float16, tag="xh")
                nc.sync.dma_start(xh, xht_r[it])
                if it == 0:
                    nc.sync.dma_start(w1sb, w1s.rearrange("p (c h) -> p c h", c=KC))
                xh8 = xin.tile([128, KC, TILE], dt.float8e4, tag="xh8")
                nc.sync.dma_start(xh8, xh8_r[it])
                if it == 1:
                    nc.sync.dma_start(w2sb, w2s.rearrange("p (c h) -> p c h", c=KC))
                return xh, xh8

            def router_logits(xh):
                """router matmuls only (PE side), zero-padded to M=128."""
                psrm = ps_rm.tile([128, TILE], dt.float32, tag="psrm")
                for kc in range(KC):
                    nc.tensor.matmul(
                        psrm, rwsb[:, kc, :], xh[:, kc, :],
                        start=(kc == 0), stop=(kc == KC - 1),
                    )
                return psrm

            def router_onehot(psrm):
                """logits^T in psum -> one-hot mask mt32[0:4] [4, TILE] fp16.

                mt32 is a [128, TILE] tile whose rows 4:128 are kept zero
                (prologue memset) so the K=128 psm expansion never sees
                garbage.  Emitted *after* the owning tile's skip adds so the
                DVE drains mm2's psum banks before starting this chain.
                """
                # alignment-safe one-hot argmax via DVE 32x32 stream transpose
                lt32s = small.tile([32, TILE], dt.float32, tag="lt32s")
                nc.scalar.copy(lt32s[0:4], psrm[0:4])
                # token-major blocks: lt32[p, 32g+r] = lt32s[r, 32g+p]
                lt32 = small.tile([32, TILE], dt.float32, tag="lt32")
                nc.vector.transpose(lt32, lt32s)
                v = lt32.rearrange("p (g r) -> p g r", r=32)
                lt_tok = small.tile([32, TILE // 32, E], dt.float32, tag="lt_tok")
                nc.vector.tensor_tensor(
                    lt_tok, v[:, :, 0:E],
                    rbsb[:, None, :].to_broadcast((32, TILE // 32, E)), add,
                )
                mxg = small.tile([32, TILE // 32], dt.float32, tag="mxg")
                nc.vector.tensor_reduce(
                    out=mxg, in_=lt_tok, axis=mybir.AxisListType.X, op=amax
                )
                mtb = small.tile([32, TILE], dt.float16, tag="mtb")
                mview = mtb.rearrange("p (g r) -> p g r", r=32)
                nc.vector.tensor_tensor(
                    mview[:, :, 0:4], lt_tok,
                    mxg[:, :, None].to_broadcast((32, TILE // 32, E)), iseq,
                )
                # back-transpose: mt32[e, t] = one_hot[t, e] for e < 4
                mt32 = small.tile([128, TILE], dt.float16, tag="mt32")
                nc.vector.transpose(mt32[0:32], mtb)
                return mt32

            # ---- prologue: spin the PE so the HAM reaches K=8/8 during the
            # DMA head, then zero the one-hot staging tiles once (their
            # padding rows are never written again).
            dummy = const.tile([128, TILE], dt.float16)
            nc.vector.memset(dummy, 0.0)
            psd = ps_h.tile([128, TILE], dt.float32, tag="psh")
            for _ in range(12):
                nc.tensor.matmul(psd, dummy[:, 0:128], dummy, start=True, stop=True)
            for _ in range(3):
                mtb0 = small.tile([32, TILE], dt.float16, tag="mtb")
                nc.vector.memset(mtb0, 0.0)
                mt0 = small.tile([128, TILE], dt.float16, tag="mt32")
                nc.vector.memset(mt0, 0.0)

            # software pipeline: router matmuls for tile n+1 issue between
            # mm1 and mm2 of tile n (PE), but their DVE argmax chain is
            # emitted after tile n's skip adds; loads run two tiles ahead.
            def psm_expand(psrm, mt32):
                """one-hot -> [128, TILE] mask in SBUF, reusing the router's
                psum bank (its logit rows were already copied to lt32s)."""
                nc.tensor.matmul(psrm, eesb, mt32, start=True, stop=True)
                psm_sb = hbuf.tile([128, TILE], dt.float16, tag="psm_sb")
                nc.scalar.copy(psm_sb, psrm)
                return psm_sb

            tiles = {0: load_tiles(0)}
            psrm0 = router_logits(tiles[0][0])
            psm_sb = psm_expand(psrm0, router_onehot(psrm0))
            tiles[1] = load_tiles(1)

            for it in range(N_TILES):
                xh, xh8 = tiles[it]

                if it + 2 < N_TILES:
                    tiles[it + 2] = load_tiles(it + 2)

                # ---- mm1: pshm = (W1^T x) * mask, mh = gelu(pshm/256 + b1)
                # experts are interleaved along H (unit j of expert e at
                # 4j+e), so the expanded one-hot (psm_sb, produced at the
                # tail of the previous tile) is one [128, TILE] tile for
                # every H-chunk.
                mh = hbuf.tile([128, KC, TILE], dt.float8e4, tag="mh")
                for hc in range(KC):
                    psh = ps_h.tile([128, TILE], dt.float32, tag="psh")
                    for k2 in range(KC // 2):
                        nc.tensor.matmul(
                            psh,
                            w1sb[:, 2 * k2 : 2 * k2 + 2, hc * 128 : (hc + 1) * 128],
                            xh8[:, 2 * k2 : 2 * k2 + 2, :],
                            start=(k2 == 0), stop=(k2 == KC // 2 - 1),
                            perf_mode=mybir.MatmulPerfMode.DoubleRow,
                        )
                    pshm = hbuf.tile([128, TILE], dt.float16, tag="pshm")
                    nc.vector.tensor_tensor(pshm, psh, psm_sb, mult)
                    # x and W1 both shipped x16: descale 1/256 here.  Masked
                    # units emit gelu(b1) constants - cancelled via b2p.
                    nc.scalar.activation(
                        mh[:, hc, :], pshm,
                        mybir.ActivationFunctionType.Gelu,
                        bias=b1sb[:, hc : hc + 1], scale=1.0 / 256.0,
                    )

                # router matmuls for the next tile sit between mm1's and
                # mm2's matmuls in the PE queue; their DVE argmax chain is
                # emitted right behind them so it runs between this tile's
                # pmasks and skip adds - finishing well before the next
                # tile's psm matmul needs mt32 (v4's tail-emitted chain made
                # the psm matmul stall ~2us at every tile boundary).
                if it + 1 < N_TILES:
                    psrm_next = router_logits(tiles[it + 1][0])
                    mt32_next = router_onehot(psrm_next)

                # ---- mm2: y^T = W2s^T @ mh + b2 via the expanded mask,
                # feature-major.  W2 chunks are stationary (resident), mh
                # streams; b2big[p,:] = b2adj[p%4,:]/32 so the K=128 matmul
                # against psm_sb injects exactly b2adj[e(t),:].
                osb = obuf.tile([128, KC, TILE], dt.float16, tag="osb")
                for dc in range(KC):
                    psy = ps_y.tile([128, TILE], dt.float32, tag="psy")
                    nc.tensor.matmul(
                        psy, b2sb[:, dc * 128 : (dc + 1) * 128], psm_sb,
                        start=True, stop=False,
                    )
                    for k2 in range(KC // 2):
                        nc.tensor.matmul(
                            psy,
                            w2sb[:, 2 * k2 : 2 * k2 + 2, dc * 128 : (dc + 1) * 128],
                            mh[:, 2 * k2 : 2 * k2 + 2, :],
                            start=False, stop=(k2 == KC // 2 - 1),
                            perf_mode=mybir.MatmulPerfMode.DoubleRow,
                        )
                    # skip connection: x^T rides the same feature-major tile
                    nc.vector.tensor_tensor(
                        osb[:, dc, :], psy, xh[:, dc, :], add,
                    )
                    if dc % 2 == 1:
                        nc.sync.dma_start(
                            out_r[it, :, dc - 1 : dc + 1, :],
                            osb[:, dc - 1 : dc + 1, :],
                        )

                # the next tile's mask expansion runs at this tile's tail:
                # its argmax chain is long done, and the ACT copy overlaps
                # the boundary so neither b2 nor the pmasks ever wait on it.
                if it + 1 < N_TILES:
                    psm_sb = psm_expand(psrm_next, mt32_next)
                del tiles[it]

    nc.compile()
    return nc


def _prep_inputs(x, router_w, router_b, w1, b1, w2, b2):
    """Host-side packing: cast/transpose; returns per-core input dicts."""
    xf = np.ascontiguousarray(np.asarray(x, dtype=F32).reshape(TOK_TOTAL, D))

    rw = np.asarray(router_w, dtype=F32)
    w1f = np.asarray(w1, dtype=F32)           # [E, D, H]
    w2f = np.asarray(w2, dtype=F32)           # [E, H, D]
    b1f = np.asarray(b1, dtype=F32)           # [E, H]
    b2f = np.asarray(b2, dtype=F32)           # [E, D]
    rb = np.asarray(router_b, dtype=F32)      # [E]
    F8 = ml_dtypes.float8_e4m3

    # router weight, zero-padded to M=128 outputs, partition-contiguous
    rwp = np.zeros((128, KC, 128), dtype=F16)
    rwp[:, :, 0:E] = rw.astype(F16).reshape(KC, 128, E).transpose(1, 0, 2)
    rwp = np.ascontiguousarray(rwp.reshape(128, KC * 128))

    # experts interleaved along the stacked hidden dim: unit j of expert e
    # lives at index 4j + e  -> the one-hot expansion pattern repeats every
    # 4 partitions, identically for each 128-row chunk.
    # Weights pre-arranged partition-contiguous: [p, c*D+m] = W[(c*128+p), m]
    w1st = w1f.transpose(1, 2, 0).reshape(D, H * E).astype(F16)
    w2st = w2f.transpose(1, 0, 2).reshape(H * E, D).astype(F16)
    w1s = np.ascontiguousarray(
        (w1st.astype(F32) * 16.0)
        .reshape(KC, 128, D).transpose(1, 0, 2).reshape(128, KC * D)).astype(F8)
    w2s = np.ascontiguousarray(
        (w2st.astype(F32) * 16.0)
        .reshape(KC, 128, D).transpose(1, 0, 2).reshape(128, KC * D)).astype(F8)
    b1all = np.ascontiguousarray(b1f.T.reshape(E * H))                    # [768]
    b1r = np.ascontiguousarray(b1all.reshape(KC, 128).T).astype(F32)      # [128, 6]
    rbt = np.ascontiguousarray(np.tile(16.0 * rb.reshape(1, E), (32, 1))).astype(F32)

    # zero-padded one-hot expansion stationary: psm[c,t] = [c%4 == e(t)]
    eep = np.zeros((128, 128), dtype=F16)
    for e in range(E):
        eep[e, e::E] = 1

    # b2 adjusted to cancel the masked units' fp8(gelu(b1)) constants:
    # device adds sum_{u masked} g8[u] w2q[u,:] = C_all - C_e per token.
    u = np.arange(H * E)
    erf1 = np.vectorize(math.erf)
    g8 = (0.5 * b1all * (1.0 + erf1(b1all / np.sqrt(2.0)))).astype(F8).astype(F32)
    w2q = w2s.reshape(128, KC, D).transpose(1, 0, 2).reshape(H * E, D).astype(F32)
    C = np.stack([(g8[u % E == e, None] * w2q[u % E == e]).sum(0) for e in range(E)])
    b2adj = 16.0 * b2f - C.sum(0)[None, :] + C                            # [E, D]
    # replicated across partitions for the K=128 injection via psm_sb
    b2p = np.ascontiguousarray(np.tile(b2adj / 32.0, (128 // E, 1))).astype(F16)

    in_maps = []
    for c in range(N_CORES):
        sl = slice(c * TOK, (c + 1) * TOK)
        xc = xf[sl]
        # feature-major 16*x stream, pre-tiled: [tile, p, (c t)]
        xht_t = np.ascontiguousarray(
            (xc * 16.0).astype(F16)
            .T.reshape(KC, 128, N_TILES, TILE)
            .transpose(2, 1, 0, 3).reshape(N_TILES, 128, KC * TILE))
        xh8_t = np.ascontiguousarray(xht_t.astype(F8))
        in_maps.append(
            {
                "xht": xht_t,
                "xh8d": xh8_t,
                "w1s": w1s,
                "w2s": w2s,
                "rwp": rwp,
                "eep": eep,
                "b2p": b2p,
                "b1r": b1r,
                "rbt": rbt,
            }
        )
    return in_maps


def _get_nc():
    global _NC_CACHE
    if _NC_CACHE is None:
        _NC_CACHE = _build_bass()
    return _NC_CACHE


def kernel(x, router_w, router_b, w1, b1, w2, b2, _trace=False, _trace_kwargs=None):
    in_maps = _prep_inputs(x, router_w, router_b, w1, b1, w2, b2)
    nc = _get_nc()
    res = run_bass_kernel_spmd(
        nc,
        in_maps,
        core_ids=list(range(N_CORES)),
        trace=_trace,
        **(_trace_kwargs or {}),
    )
    outs = []
    for r in res.results:
        o = r["out"].reshape(N_TILES, 128, KC, TILE)     # [it, p, dc, t] fp16
        # y^T feature-major -> token-major [TOK, D], descale by 16
        yt = o.transpose(0, 3, 2, 1).reshape(TOK, D)     # [it*t, dc*128(=d)]
        outs.append(yt.astype(np.float32) * (1.0 / 16.0))
    full = np.concatenate(outs, axis=0).reshape(B, S, D)
    if _trace:
        kernel.last_results = res
    return full
